# revision 30
# baseline (speedup 1.0000x reference)
"""GAT (graph attention) layer on 8 TRN2 NeuronCores.

Algorithm (mathematically equal to the reference):
  proj = in_feat @ W_proj;  src_s = proj @ A_src;  tau = proj @ A_tgt
  per edge e=(s,t):  score_e = exp(leakyrelu(src_s[s] + tau[t]) - SHIFT)
  out[t] = (sum_e score_e * proj[s]) / (sum_e score_e + eps) + bias

The reference subtracts the global max of the pre-activation scores before
exp(); since numerator and denominator scale identically, any constant shift
yields the same output.  Input scales are fixed by the problem spec
(randn fills, Xavier scaling), so scores lie in ~[-11, 11]; SHIFT=16 keeps
exp() comfortably in fp32 range.

Sharding: edges are sharded by TARGET node, so each core owns a disjoint
output slice and no collective is needed.  dma_gather indices are int16, so
node ids are split at 32768: each core gets 4096 "lo" targets (32 blocks of
128) and 2154 "hi" targets (17 blocks), and the projection/score tables are
split into lo/hi halves.  Every core computes the full projection table
(redundantly), writes packed rows [proj bf16 | src_score f32 | tau f32] to
HBM, gathers rows by edge source id, and accumulates both segment sums
(weighted features + softmax denominators) in PSUM with one-hot matmuls over
128-target blocks.
"""
import sys
sys.path.insert(0, "/opt/trn_rl_repo")
import numpy as np

import concourse.bass as bass
import concourse.bacc as bacc
import concourse.mybir as mybir
import concourse.tile as tile
from concourse._compat import cdiv
from concourse.library_config import mlp

P = 128
N_NODES = 50000
N_CORES = 8
SPLIT = 32768                       # int16-safe table split
LO_TPN = SPLIT // N_CORES           # 4096 lo targets per core
HI_TPN = (N_NODES - SPLIT) // N_CORES  # 2154 hi targets per core
LO_NBLK = LO_TPN // P               # 32
HI_NBLK = cdiv(HI_TPN, P)           # 17
NBLK = LO_NBLK + HI_NBLK            # 49
NPAD = cdiv(N_NODES, P) * P         # 50048
NT_NODE = NPAD // P                 # 391
LO_ROWS = SPLIT                     # table_lo rows (= node tiles 0..255)
HI_ROWS = NPAD - SPLIT              # 17280 (tiles 256..390)
D = 128
H = 4
SHIFT = 16.0
EPS = 1e-16

_cache = {}

# tunables (ablation sweeps poke these before _build)
CFG = {
    "chunk": 18,
    "expand_on_act": True,   # ACT writes 32x-expanded scores (DVE 2x mul)
    "writes_on_scalar": "alt",  # "alt"|"pool"|True|False: phase-1 write queue
    "swdge_queues": 1,
    "slab": 24,
    "pk": 8,
    "p1copy_act": False,
    "ep_batch": 1,
    "p1ps_bufs": 1,
    "acc_bufs": 3,
    "wk_bufs": 4,
    "g_bufs": 3,
}


def _build(k_lo, k_hi, chunk=None):
    if chunk is None:
        chunk = CFG["chunk"]
    nc = bacc.Bacc("TRN2", target_bir_lowering=False, debug=False,
                   num_swdge_queues=CFG["swdge_queues"])
    f32, bf16 = mybir.dt.float32, mybir.dt.bfloat16
    i16 = mybir.dt.int16

    T_B = k_lo + k_hi
    NIDX = T_B * P
    IW = T_B * 8                    # int16 idx cols per block (wrapped /16)
    T_TOT = NBLK * T_B

    xT_d = nc.dram_tensor("xT", [P, NPAD], bf16, kind="ExternalInput")
    W_d = nc.dram_tensor("W", [P, D], bf16, kind="ExternalInput")
    WT_d = nc.dram_tensor("WT", [P, D], bf16, kind="ExternalInput")
    A_d = nc.dram_tensor("A", [P, 2 * H], bf16, kind="ExternalInput")
    bias_d = nc.dram_tensor("bias", [1, D], f32, kind="ExternalInput")
    srcidx_d = nc.dram_tensor("srcidx", [P, NBLK * IW], i16, kind="ExternalInput")
    tgtidx_d = nc.dram_tensor("tgtidx", [P, NBLK * IW], i16, kind="ExternalInput")
    tgtinb_d = nc.dram_tensor("tgtinb", [P, T_TOT], i16, kind="ExternalInput")
    out_d = nc.dram_tensor("out", [NBLK * P, D], f32, kind="ExternalOutput")

    # packed row: [proj 128 bf16 | src_s 4 f32 | tau 4 f32 | pad] = 128 f32
    t1lo = nc.dram_tensor("t1lo", [LO_ROWS, 128], f32)
    t1hi = nc.dram_tensor("t1hi", [HI_ROWS, 128], f32)
    # tau row: [tau 4 f32 | pad] = 64 f32 (256B dma_gather minimum)
    t2lo = nc.dram_tensor("t2lo", [LO_ROWS, 64], f32)
    t2hi = nc.dram_tensor("t2hi", [HI_ROWS, 64], f32)

    with tile.TileContext(nc) as tc:
        with (
            tc.tile_pool(name="const", bufs=1) as cp,
            tc.tile_pool(name="p1x", bufs=2) as p1x,
            tc.tile_pool(name="p1o", bufs=CFG.get("p1o_bufs", 4)) as p1o,
            tc.tile_pool(name="p1ps", bufs=CFG["p1ps_bufs"], space="PSUM") as p1ps,
            tc.tile_pool(name="initps", bufs=1, space="PSUM") as initps,
            tc.tile_pool(name="g", bufs=CFG["g_bufs"]) as g,
            tc.tile_pool(name="wk", bufs=CFG["wk_bufs"]) as wk,
            tc.tile_pool(name="acc", bufs=CFG["acc_bufs"], space="PSUM") as accp,
            tc.tile_pool(name="ep", bufs=CFG.get("ep_bufs", 2)) as ep,
        ):
            nc.gpsimd.load_library(mlp)
            # ---- constants ----
            # iota_qB[p, q*chunk + j] = q  (q-major so S-build APs stay packed)
            iota_qB = cp.tile([P, P * chunk], i16)
            nc.gpsimd.iota(iota_qB[:], pattern=[[1, P], [0, chunk]], base=0,
                           channel_multiplier=0)
            nshift = cp.tile([P, 1], f32)
            nc.gpsimd.memset(nshift[:], -SHIFT)
            epsb = cp.tile([P, 1], f32)
            nc.gpsimd.memset(epsb[:], EPS)
            srcidx = cp.tile([P, NBLK * IW], i16)
            tgtidx = cp.tile([P, NBLK * IW], i16)
            tgtinb = cp.tile([P, T_TOT], i16)
            nc.sync.dma_start(srcidx[:], srcidx_d[:])
            nc.sync.dma_start(tgtidx[:], tgtidx_d[:])
            nc.sync.dma_start(tgtinb[:], tgtinb_d[:])

            # W_ext = [W | W @ A | pad], bf16 (psum matmul slices stay
            # 1KB bank-aligned with 256 cols)
            W_ext = cp.tile([P, 256], bf16)
            nc.gpsimd.memset(W_ext[:], 0.0)
            nc.sync.dma_start(W_ext[:, :D], W_d[:])
            WT_sb = cp.tile([P, D], bf16)
            A_sb = cp.tile([P, 2 * H], bf16)
            nc.sync.dma_start(WT_sb[:], WT_d[:])
            nc.sync.dma_start(A_sb[:], A_d[:])
            wa_ps = initps.tile([P, 2 * H], f32, tag="init")
            nc.tensor.matmul(out=wa_ps[:], lhsT=WT_sb[:], rhs=A_sb[:],
                             start=True, stop=True)
            nc.vector.tensor_copy(out=W_ext[:, D:D + 2 * H], in_=wa_ps[:])

            # bias broadcast to all partitions
            ones_row = cp.tile([1, P], f32)
            nc.gpsimd.memset(ones_row[:], 1.0)
            bias_row = cp.tile([1, D], f32)
            nc.sync.dma_start(bias_row[:], bias_d[:])
            bias_ps = initps.tile([P, D], f32, tag="init")
            nc.tensor.matmul(out=bias_ps[:], lhsT=ones_row[:], rhs=bias_row[:],
                             start=True, stop=True)
            bias_mat = cp.tile([P, D], f32)
            nc.vector.tensor_copy(out=bias_mat[:], in_=bias_ps[:])

            # ---- phase 1: projection + scores -> packed tables ----
            PK = CFG["pk"]
            SLAB = CFG["slab"]  # node tiles per input DMA
            CW = 256  # psum cols per node tile (fp32r wants >=256 moving)
            slabs = {}
            for s in range(0, NT_NODE, SLAB):
                w = min(SLAB, NT_NODE - s)
                xs = p1x.tile([P, SLAB * P], bf16, tag="xslab")
                nc.sync.dma_start(xs[:, :w * P], xT_d[:, s * P:(s + w) * P])
                slabs[s] = xs
            p1_groups = [(s, min(s + PK, stop))
                         for lo, stop in ((0, 256), (256, NT_NODE))
                         for s in range(lo, stop, PK)]
            for base, stop in p1_groups:
                k = stop - base
                ps = p1ps.tile([P, PK * CW], f32, tag="p1")
                for j in range(k):
                    nt = base + j
                    xs = slabs[(nt // SLAB) * SLAB]
                    o = (nt % SLAB) * P
                    nc.tensor.matmul(out=ps[:, j * CW:(j + 1) * CW],
                                     lhsT=xs[:, o:o + P], rhs=W_ext[:],
                                     start=True, stop=True)
                ps_r = ps[:].rearrange("p (j c) -> p j c", j=PK)[:, :k, :]
                row_sb = p1o.tile([P, PK * 72], f32, tag="rows")
                row_r = row_sb[:].rearrange("p (j c) -> p j c", j=PK)[:, :k, :]
                # proj -> bf16 into cols [0:64) (f32 units) of each 72-col row
                ceng = nc.scalar if (CFG["p1copy_act"] and (base // PK) % 2 == 0) else nc.vector
                # proj stored head-interleaved (col r*4+h) so the weighted
                # multiply's score operand keeps a packed last dim (2x DVE)
                if ceng is nc.scalar:
                    nc.scalar.activation(
                        out=row_r[:, :, 0:64].bitcast(bf16)
                            .rearrange("p j (r h) -> p j r h", h=H),
                        in_=ps_r[:, :, 0:D].rearrange(
                            "p j (h r) -> p j r h", h=H),
                        func=mybir.ActivationFunctionType.Copy)
                else:
                    nc.vector.tensor_copy(
                        out=row_r[:, :, 0:64].bitcast(bf16)
                            .rearrange("p j (r h) -> p j r h", h=H),
                        in_=ps_r[:, :, 0:D].rearrange(
                            "p j (h r) -> p j r h", h=H))
                # src_s | tau (f32) into cols [64:72)
                nc.vector.tensor_copy(
                    out=row_r[:, :, 64:72],
                    in_=ps_r[:, :, D:D + 8])
                # write packed rows + tau table
                r0 = base * P
                if base < 256:
                    t1, t2, off = t1lo, t2lo, r0
                else:
                    t1, t2, off = t1hi, t2hi, r0 - SPLIT
                wmode = CFG["writes_on_scalar"]
                if wmode == "pool":
                    weng = nc.gpsimd
                elif wmode == "altsp":
                    weng = nc.sync if (base // PK) % 2 == 0 else nc.gpsimd
                elif wmode == "alt":
                    weng = nc.scalar if (base // PK) % 2 == 0 else nc.sync
                elif wmode:
                    weng = nc.scalar
                else:
                    weng = nc.sync
                weng.dma_start(
                    t1[off:off + k * P, 0:72].rearrange(
                        "(j p) c -> p j c", p=P),
                    row_r[:, :, :])
                weng.dma_start(
                    t2[off:off + k * P, 0:4].rearrange(
                        "(j p) c -> p j c", p=P),
                    row_r[:, :, 68:72])

            # ---- phase 2: edge processing per 128-target block ----
            NCH = cdiv(T_B, chunk)
            for b in range(NBLK):
                is_lo = b < LO_NBLK
                t1a, t1b = t1lo, t1hi
                t2h = t2lo if is_lo else t2hi
                acc = accp.tile([P, D + H], f32, tag="acc")
                gi0 = b * IW
                rows = g.tile([P, T_B * 128], f32, tag="grow")
                taut = g.tile([P, T_B * 64], f32, tag="gtau")
                if k_lo:
                    nc.gpsimd.dma_gather(
                        rows[:, :k_lo * 128].rearrange("p (k c) -> p k c", k=k_lo),
                        t1a[:], srcidx[:, gi0:gi0 + k_lo * 8],
                        k_lo * P, k_lo * P, 128, single_packet=False)
                if k_hi:
                    nc.gpsimd.dma_gather(
                        rows[:, k_lo * 128:].rearrange("p (k c) -> p k c", k=k_hi),
                        t1b[:], srcidx[:, gi0 + k_lo * 8:gi0 + IW],
                        k_hi * P, k_hi * P, 128, single_packet=False)
                nc.gpsimd.dma_gather(
                    taut[:].rearrange("p (k c) -> p k c", k=T_B),
                    t2h[:], tgtidx[:, gi0:gi0 + IW],
                    NIDX, NIDX, 64, single_packet=False)

                rows_r = rows[:].rearrange("p (j c) -> p j c", j=T_B)
                taut_r = taut[:].rearrange("p (j c) -> p j c", j=T_B)
                # per-block score chain: x = src_s + tau; y = max(x, 0.2x)
                xb = wk.tile([P, T_B * H], f32, tag="xb")
                ab = wk.tile([P, T_B * H], f32, tag="ab")
                nc.vector.tensor_tensor(
                    out=xb[:].rearrange("p (j h) -> p j h", j=T_B),
                    in0=rows_r[:, :, 64:68], in1=taut_r[:, :, 0:4],
                    op=mybir.AluOpType.add)
                nc.vector.tensor_scalar(
                    out=ab[:], in0=xb[:], scalar1=0.2, scalar2=None,
                    op0=mybir.AluOpType.mult)
                nc.vector.tensor_tensor(
                    out=ab[:], in0=ab[:], in1=xb[:], op=mybir.AluOpType.max)
                for ch in range(NCH):
                    t0 = ch * chunk
                    B = min(chunk, T_B - t0)
                    gt0 = b * T_B + t0
                    # S stored q-major: S[p, q*chunk + j] so every DVE operand
                    # keeps a packed (step-1) last dim -> 2x DVE mode
                    S = wk.tile([P, P * chunk], bf16, tag="S")
                    wide = wk.tile([P, chunk * (D + H)], bf16, tag="wide")
                    wide_r = wide[:].rearrange("p (j c) -> p j c", j=chunk)
                    S_r = S[:].rearrange("p (q j) -> p q j", q=P)
                    # one-hot S[e, q, j] = (tgt_in_block[e, j] == q)
                    nc.vector.tensor_tensor(
                        out=S_r[:, :, :B],
                        in0=tgtinb[:, gt0:gt0 + B].unsqueeze(1)
                            .to_broadcast([P, P, B]),
                        in1=iota_qB[:].rearrange("p (q j) -> p q j", q=P)[:, :, :B],
                        op=mybir.AluOpType.is_equal)
                    # score (unexpanded) into wide for the denominator columns
                    nc.scalar.activation(
                        out=wide_r[:, :B, D:],
                        in_=ab[:, t0 * H:(t0 + B) * H].rearrange(
                            "p (j h) -> p j h", j=B),
                        func=mybir.ActivationFunctionType.Exp,
                        bias=nshift[:])
                    # weighted = proj_bf16 * score; proj is head-interleaved
                    # so score's broadcast lands on a non-last dim (2x DVE)
                    nc.vector.tensor_tensor(
                        out=wide_r[:, :B, :D].rearrange(
                            "p j (r h) -> p j r h", h=H),
                        in0=rows_r[:, t0:t0 + B, 0:64].bitcast(bf16)
                            .rearrange("p j (r h) -> p j r h", h=H),
                        in1=wide_r[:, :B, D:].unsqueeze(2)
                            .to_broadcast([P, B, 32, H]),
                        op=mybir.AluOpType.mult)
                    for j in range(B):
                        gidx = t0 + j
                        nc.tensor.matmul(
                            out=acc[:],
                            lhsT=S_r[:, :, j],
                            rhs=wide[:, j * (D + H):(j + 1) * (D + H)],
                            start=(gidx == 0), stop=(gidx == T_B - 1))
                # epilogue: out = num / (den + eps) + bias
                den = ep.tile([P, H], f32, tag="den")
                nc.scalar.activation(out=den[:], in_=acc[:, D:],
                                     func=mybir.ActivationFunctionType.Copy,
                                     bias=float(EPS))
                recip = ep.tile([P, H], f32, tag="recip")
                nc.vector.reciprocal(recip[:], den[:])
                out_sb = ep.tile([P, D], f32, tag="outsb")
                for h in range(H):
                    nc.scalar.activation(
                        out=out_sb[:, h * 32:(h + 1) * 32],
                        in_=acc[:, :D].rearrange(
                            "p (q h) -> p h q", h=H)[:, h, :],
                        func=mybir.ActivationFunctionType.Copy,
                        scale=recip[:, h:h + 1])
                nc.vector.tensor_tensor(
                    out=out_sb[:], in0=out_sb[:], in1=bias_mat[:],
                    op=mybir.AluOpType.add)
                nc.sync.dma_start(out_d[b * P:(b + 1) * P, :], out_sb[:])

    nc.compile()
    return nc


def _wrap16(seg):
    """dma_gather idx layout: entry i at [i%16, i//16], replicated to the
    8 groups of 16 partitions."""
    n = len(seg)
    w = seg.reshape(n // 16, 16).T  # [16, n/16]
    return np.tile(w, (8, 1))


def _prep_host(in_feat, edge_ind, W_proj, a_src, a_tgt, bias):
    src = np.asarray(edge_ind[0]).astype(np.int64)
    tgt = np.asarray(edge_ind[1]).astype(np.int64)

    import ml_dtypes
    bfd = ml_dtypes.bfloat16
    xT = np.zeros((P, NPAD), bfd)
    xT[:, :N_NODES] = np.asarray(in_feat, np.float32).T.astype(bfd)
    W = np.ascontiguousarray(np.asarray(W_proj, np.float32).astype(bfd))
    WT = np.ascontiguousarray(W.T)
    A = np.zeros((P, 2 * H), bfd)
    a_src = np.asarray(a_src, np.float32)
    a_tgt = np.asarray(a_tgt, np.float32)
    for h in range(H):
        A[h * 32:(h + 1) * 32, h] = a_src[0, h]
        A[h * 32:(h + 1) * 32, H + h] = a_tgt[0, h]
    bias_row = np.asarray(bias, np.float32).reshape(1, D)

    # assign each edge to (core, block, in-block target slot); within each
    # core's lo/hi half, targets are packed into blocks balancing the
    # lo-src and hi-src edge counts (smaller uniform tiles-per-block)
    is_lo = tgt < SPLIT
    core = np.where(is_lo, tgt // LO_TPN, (tgt - SPLIT) // HI_TPN)
    src_is_lo = src < SPLIT
    deg_lo = np.bincount(tgt[src_is_lo], minlength=N_NODES).astype(np.int64)
    deg_hi = np.bincount(tgt[~src_is_lo], minlength=N_NODES).astype(np.int64)

    blk_of = np.zeros(N_NODES, np.int32)   # block index within core
    tin_of = np.zeros(N_NODES, np.int32)   # slot within block
    for c in range(N_CORES):
        for base, n_t, b0, nb in (
                (c * LO_TPN, LO_TPN, 0, LO_NBLK),
                (SPLIT + c * HI_TPN, HI_TPN, LO_NBLK, HI_NBLK)):
            ids = np.arange(base, base + n_t)
            order = np.argsort(-(deg_lo[ids] + deg_hi[ids]), kind="stable")
            loads_l = np.zeros(nb, np.int64)
            loads_h = np.zeros(nb, np.int64)
            fill = np.zeros(nb, np.int32)
            for t in ids[order]:
                cand = np.nonzero(fill < P)[0]
                j = cand[np.argmin(np.maximum(loads_l[cand] + deg_lo[t],
                                              loads_h[cand] + deg_hi[t])
                                   + 0.001 * fill[cand])]
                blk_of[t] = b0 + j
                tin_of[t] = fill[j]
                fill[j] += 1
                loads_l[j] += deg_lo[t]
                loads_h[j] += deg_hi[t]
    blk = blk_of[tgt]
    tin = tin_of[tgt]

    # per (core, block): count lo-src and hi-src edges
    key = (core * NBLK + blk).astype(np.int64)
    n_lo_e = np.bincount(key[src_is_lo], minlength=N_CORES * NBLK)
    n_hi_e = np.bincount(key[~src_is_lo], minlength=N_CORES * NBLK)
    k_lo = max(1, cdiv(int(n_lo_e.max()), P))
    k_hi = max(1, cdiv(int(n_hi_e.max()), P))
    T_B = k_lo + k_hi
    IW = T_B * 8

    core_inputs = []
    shared = {"xT": xT, "W": W, "WT": WT, "A": A, "bias": bias_row}
    ctg_all = np.where(is_lo, tgt, tgt - SPLIT)  # half-table row of target
    out_perm = np.zeros((N_CORES, NBLK * P), np.int64)  # out row -> node id
    for c in range(N_CORES):
        ids_lo = np.arange(c * LO_TPN, (c + 1) * LO_TPN)
        ids_hi = np.arange(SPLIT + c * HI_TPN, SPLIT + (c + 1) * HI_TPN)
        perm = np.full(NBLK * P, -1, np.int64)
        for t in np.concatenate([ids_lo, ids_hi]):
            perm[blk_of[t] * P + tin_of[t]] = t
        out_perm[c] = perm
        m = core == c
        cs, cb, ct, clo = src[m], blk[m], tin[m], src_is_lo[m]
        sidx = np.zeros((NBLK, T_B * P), np.int16)
        gidx = np.zeros((NBLK, T_B * P), np.int16)
        tinb = np.full((NBLK, T_B * P), -1, np.int16)
        ctg = ctg_all[m]  # half-table row id of each edge's target
        for b in range(NBLK):
            mb_ = cb == b
            lo_sel = mb_ & clo
            hi_sel = mb_ & ~clo
            nl, nh = int(lo_sel.sum()), int(hi_sel.sum())
            sidx[b, :nl] = cs[lo_sel].astype(np.int16)
            sidx[b, k_lo * P:k_lo * P + nh] = (cs[hi_sel] - SPLIT).astype(np.int16)
            gidx[b, :nl] = ctg[lo_sel].astype(np.int16)
            gidx[b, k_lo * P:k_lo * P + nh] = ctg[hi_sel].astype(np.int16)
            tinb[b, :nl] = ct[lo_sel].astype(np.int16)
            tinb[b, k_lo * P:k_lo * P + nh] = ct[hi_sel].astype(np.int16)
        # wrap idx arrays for dma_gather (segment-wise)
        s16 = np.zeros((P, NBLK * IW), np.int16)
        g16 = np.zeros((P, NBLK * IW), np.int16)
        for b in range(NBLK):
            s16[:, b * IW:b * IW + k_lo * 8] = _wrap16(sidx[b, :k_lo * P])
            s16[:, b * IW + k_lo * 8:(b + 1) * IW] = _wrap16(sidx[b, k_lo * P:])
            g16[:, b * IW:(b + 1) * IW] = _wrap16(gidx[b])
        tinb_t = np.ascontiguousarray(tinb.reshape(NBLK * T_B, P).T)
        core_inputs.append({**shared,
                            "srcidx": s16, "tgtidx": g16, "tgtinb": tinb_t})
    return k_lo, k_hi, core_inputs, out_perm


def kernel(in_feat, edge_ind, edge_len, W_proj, a_src, a_tgt, bias):
    k_lo, k_hi, core_inputs, out_perm = _prep_host(in_feat, edge_ind, W_proj,
                                                   a_src, a_tgt, bias)
    if (k_lo, k_hi) not in _cache:
        _cache[(k_lo, k_hi)] = _build(k_lo, k_hi)
    nc = _cache[(k_lo, k_hi)]

    from concourse.bass_utils import run_bass_kernel_spmd
    res = run_bass_kernel_spmd(nc, core_inputs, list(range(N_CORES)))

    out = np.zeros((N_NODES, D), np.float32)
    for c in range(N_CORES):
        o = res.results[c]["out"]
        valid = out_perm[c] >= 0
        out[out_perm[c][valid]] = o[valid]
    return out
